# revision 4
# baseline (speedup 1.0000x reference)
"""Trainium2 Bass kernel for a transformer decoder layer (self-attn + cross-attn + FFN,
3 LayerNorms). Data-parallel over batch: 8 batch elements -> 8 NeuronCores, no collectives.

Per-core dataflow (one batch element, S=512, D=1024, H=16, HD=64, DFF=4096):
  - Activations live feature-major in SBUF: X^T [D, T] as tiles [128, D/128, T].
  - Projections: out X'^T[mc] = sum_kc W[kc,mc].T @ X^T[kc]  (weights stationary).
  - Scores computed transposed: s^T[s,t] = k_h^T(.,s).T @ q_h^T  (K=HD=64).
  - Softmax without max-subtraction (scores are O(1); masked entries get -1e5 -> exp==0).
    Denominator via a ones-column prepended to V in the AV matmul (psum row 0).
  - AV: bU^T[hd,t] = V_aug[s,:].T @ exp(s^T), normalize with partition-broadcast 1/denom.
  - LayerNorm feature-major: stats over partitions via ones-vector matmuls, apply with
    partition-broadcast mean/rstd.
All host-side reshapes/transposes (inputs, weights, output) are in kernel() below.
"""
import numpy as np

B, S, D, H, HD, DFF = 8, 512, 1024, 16, 64, 4096
KC = D // 128    # 8 feature chunks
SC = S // 128    # 4 sequence chunks
FC = DFF // 128  # 32 ffn chunks
QSCALE = float(1.0 / (np.sqrt(np.float32(1024.0)) + 1e-8))
NEGBIG = -1.0e5
LN_EPS = 1e-3

_CACHE = {}


def _build_nc():
    import concourse.mybir as mybir
    import concourse.tile as tile
    from concourse import bacc

    F32 = mybir.dt.float32
    AF = mybir.ActivationFunctionType
    OP = mybir.AluOpType

    nc = bacc.Bacc("TRN2", target_bir_lowering=False, debug=False, num_devices=8)

    # ---- DRAM I/O ----
    yzT = nc.dram_tensor("yzT", [128, KC, S], F32, kind="ExternalInput")
    xzT = nc.dram_tensor("xzT", [128, KC, S], F32, kind="ExternalInput")
    maskT_d = nc.dram_tensor("maskT", [128, SC, S], F32, kind="ExternalInput")
    padb_d = nc.dram_tensor("padb", [128, SC], F32, kind="ExternalInput")
    wdec = {}
    for nm in ("wq_s", "wk_s", "wo_s", "wq_c", "wk_c", "wo_c"):
        wdec[nm] = nc.dram_tensor(nm, [KC, 128, KC, 128], F32, kind="ExternalInput")
    for nm in ("wv_s", "wv_c"):
        wdec[nm] = nc.dram_tensor(nm, [KC, 128, D], F32, kind="ExternalInput")
    w1_d = nc.dram_tensor("w1", [FC, 128, KC, 128], F32, kind="ExternalInput")
    w2_d = nc.dram_tensor("w2", [KC, 128, FC, 128], F32, kind="ExternalInput")
    b1_d = nc.dram_tensor("b1", [128, FC], F32, kind="ExternalInput")
    b2_d = nc.dram_tensor("b2", [128, KC], F32, kind="ExternalInput")
    lnp = {}
    for nm in ("g0", "be0", "g1", "be1", "g2", "be2"):
        lnp[nm] = nc.dram_tensor(nm, [128, KC], F32, kind="ExternalInput")
    outT_d = nc.dram_tensor("outT", [128, KC, S], F32, kind="ExternalOutput")

    with tile.TileContext(nc) as tc:
        with tc.tile_pool(name="persist", bufs=1) as persist:
            # ---- constants / persistent activations ----
            ones_t = persist.tile([128, 1], F32, tag="ones", name="ones_t")
            nc.vector.memset(ones_t[:], 1.0)
            eps_t = persist.tile([1, 1], F32, tag="eps", name="eps_t")
            nc.vector.memset(eps_t[:], LN_EPS)
            padb_t = persist.tile([128, SC], F32, tag="padb", name="padb_t")
            nc.sync.dma_start(out=padb_t[:], in_=padb_d[:, :])
            b1_t = persist.tile([128, FC], F32, tag="b1", name="b1_t")
            nc.sync.dma_start(out=b1_t[:], in_=b1_d[:, :])
            b2_t = persist.tile([128, KC], F32, tag="b2", name="b2_t")
            nc.sync.dma_start(out=b2_t[:], in_=b2_d[:, :])
            ln_t = {}
            for nm in lnp:
                ln_t[nm] = persist.tile([128, KC], F32, tag=nm, name=nm + "_t")
                nc.sync.dma_start(out=ln_t[nm][:], in_=lnp[nm][:, :])

            yzT_t = persist.tile([128, KC, S], F32, tag="yzT", name="yzT_t")
            nc.sync.dma_start(out=yzT_t[:], in_=yzT[:, :, :])
            xzT_t = persist.tile([128, KC, S], F32, tag="xzT", name="xzT_t")
            nc.sync.dma_start(out=xzT_t[:], in_=xzT[:, :, :])
            o1T_t = persist.tile([128, KC, S], F32, tag="o1T", name="o1T_t")
            o2T_t = persist.tile([128, KC, S], F32, tag="o2T", name="o2T_t")
            resT_t = persist.tile([128, KC, S], F32, tag="resT", name="resT_t")

            # ---------------- helpers ----------------
            def emit_attention(idx, qsrcT, kvT, wq, wk, wv, wo, is_self, residT):
                """Writes resT_t = attn_out + residT (feature-major chunks)."""
                with tc.tile_pool(name=f"attn{idx}", bufs=1) as ap, \
                     tc.tile_pool(name=f"attn{idx}_a", bufs=4) as apool, \
                     tc.tile_pool(name=f"attn{idx}_w", bufs=2) as wpool, \
                     tc.tile_pool(name=f"attn{idx}_s", bufs=2) as spool, \
                     tc.tile_pool(name=f"attn{idx}_d", bufs=2) as dnmp, \
                     tc.tile_pool(name=f"attn{idx}_pm", bufs=4, space="PSUM") as pmm, \
                     tc.tile_pool(name=f"attn{idx}_pav", bufs=2, space="PSUM") as pavp:
                    QT = ap.tile([128, KC, S], F32, tag="QT", name=f"QT{idx}")
                    KT = ap.tile([128, KC, S], F32, tag="KT", name=f"KT{idx}")
                    Vt = ap.tile([128, SC, H, HD + 1], F32, tag="Vt", name=f"Vt{idx}")
                    bT = ap.tile([128, KC, S], F32, tag="bT", name=f"bT{idx}")

                    # Q projection (scaled on evict) and K projection
                    for which, wsrc, dst in (("q", wq, QT), ("k", wk, KT)):
                        for mc in range(KC):
                            wt = wpool.tile([128, KC, 128], F32, tag="wst",
                                            name=f"w{which}{idx}_{mc}")
                            nc.sync.dma_start(out=wt[:], in_=wsrc[mc, :, :, :])
                            pq = pmm.tile([128, S], F32, tag="pmm",
                                          name=f"p{which}{idx}_{mc}")
                            for kc in range(KC):
                                nc.tensor.matmul(pq[:], wt[:, kc, :], qsrcT[:, kc, :],
                                                 start=(kc == 0), stop=(kc == KC - 1))
                            if which == "q":
                                nc.scalar.activation(out=dst[:, mc, :], in_=pq[:],
                                                     func=AF.Copy, scale=QSCALE)
                            else:
                                nc.vector.tensor_copy(out=dst[:, mc, :], in_=pq[:])

                    # V projection: normal layout [s, (h hd)] with ones column at 64
                    for j in range(SC):
                        nc.vector.memset(Vt[:, j, :, HD:HD + 1], 1.0)
                    for nh in range(2):
                        pvs = []
                        for j in range(SC):
                            pv = pmm.tile([128, S], F32, tag="pmm",
                                          name=f"pv{idx}_{nh}_{j}")
                            pvs.append(pv)
                        for kc in range(KC):
                            wvt = wpool.tile([128, 512], F32, tag="wmv",
                                             name=f"wv{idx}_{nh}_{kc}")
                            nc.sync.dma_start(
                                out=wvt[:], in_=wv[kc, :, 512 * nh:512 * (nh + 1)])
                            for j in range(SC):
                                nc.tensor.matmul(
                                    pvs[j][:], kvT[:, kc, 128 * j:128 * (j + 1)],
                                    wvt[:],
                                    start=(kc == 0), stop=(kc == KC - 1))
                        for j in range(SC):
                            # psum [128, 512] -> V[:, j, 8nh:8nh+8, 0:64]
                            nc.vector.tensor_copy(
                                out=Vt[:, j, 8 * nh:8 * (nh + 1), 0:HD],
                                in_=pvs[j][:])

                    # per-head scores + AV + normalize
                    for h in range(H):
                        mc, off = h // 2, 64 * (h % 2)
                        pav = pavp.tile([HD + 1, S], F32, tag="pav",
                                        name=f"pav{idx}_{h}")
                        for j in range(SC):
                            pst = pmm.tile([128, S], F32, tag="pmm",
                                           name=f"ps{idx}_{h}_{j}")
                            nc.tensor.matmul(
                                pst[:],
                                KT[off:off + 64, mc, 128 * j:128 * (j + 1)],
                                QT[off:off + 64, mc, :],
                                start=True, stop=True)
                            aT = apool.tile([128, S], F32, tag="aT",
                                            name=f"aT{idx}_{h}_{j}")
                            if is_self:
                                nc.vector.tensor_add(out=aT[:], in0=pst[:],
                                                     in1=maskT_t[:, j, :])
                                nc.scalar.activation(out=aT[:], in_=aT[:], func=AF.Exp)
                            else:
                                nc.scalar.activation(out=aT[:], in_=pst[:], func=AF.Exp,
                                                     bias=padb_t[:, j:j + 1])
                            nc.tensor.matmul(pav[:], Vt[:, j, h, :], aT[:],
                                             start=(j == 0), stop=(j == SC - 1))
                        dnm = dnmp.tile([1, S], F32, tag="dnm", name=f"dnm{idx}_{h}")
                        nc.vector.tensor_copy(out=dnm[:], in_=pav[HD:HD + 1, :])
                        rb = spool.tile([64, S], F32, tag="rb", name=f"rb{idx}_{h}")
                        nc.gpsimd.partition_broadcast(rb[:], dnm[:])
                        nc.vector.reciprocal(out=rb[:], in_=rb[:])
                        nc.vector.tensor_mul(out=bT[off:off + 64, mc, :],
                                             in0=pav[0:HD, :], in1=rb[:])

                    # output projection + residual add
                    for mc in range(KC):
                        wt = wpool.tile([128, KC, 128], F32, tag="wst",
                                        name=f"wo{idx}_{mc}")
                        nc.sync.dma_start(out=wt[:], in_=wo[mc, :, :, :])
                        po = pmm.tile([128, S], F32, tag="pmm", name=f"po{idx}_{mc}")
                        for kc in range(KC):
                            nc.tensor.matmul(po[:], wt[:, kc, :], bT[:, kc, :],
                                             start=(kc == 0), stop=(kc == KC - 1))
                        nc.vector.tensor_add(out=resT_t[:, mc, :], in0=po[:],
                                             in1=residT[:, mc, :])

            def emit_ln(idx, g_ap, b_ap, outT):
                """LayerNorm over features of resT_t -> outT ([128, KC, S] tile or
                None to stream to DRAM output)."""
                with tc.tile_pool(name=f"ln{idx}_ps", bufs=2, space="PSUM") as pstat, \
                     tc.tile_pool(name=f"ln{idx}_st", bufs=2) as stage, \
                     tc.tile_pool(name=f"ln{idx}_sm", bufs=1) as lnsm:
                    psm = pstat.tile([1, S], F32, tag="psm", name=f"psm{idx}")
                    for kc in range(KC):
                        nc.tensor.matmul(psm[:], ones_t[:], resT_t[:, kc, :],
                                         start=(kc == 0), stop=(kc == KC - 1))
                    pss = pstat.tile([1, S], F32, tag="psm", name=f"pss{idx}")
                    for kc in range(KC):
                        xsq = stage.tile([128, S], F32, tag="xsq",
                                         name=f"xsq{idx}_{kc}")
                        nc.scalar.activation(out=xsq[:], in_=resT_t[:, kc, :],
                                             func=AF.Square)
                        nc.tensor.matmul(pss[:], ones_t[:], xsq[:],
                                         start=(kc == 0), stop=(kc == KC - 1))
                    mrow = lnsm.tile([1, S], F32, tag="mrow", name=f"mrow{idx}")
                    nc.scalar.activation(out=mrow[:], in_=psm[:], func=AF.Copy,
                                         scale=1.0 / D)
                    erow = lnsm.tile([1, S], F32, tag="erow", name=f"erow{idx}")
                    nc.scalar.activation(out=erow[:], in_=pss[:], func=AF.Copy,
                                         scale=1.0 / D)
                    vrow = lnsm.tile([1, S], F32, tag="vrow", name=f"vrow{idx}")
                    nc.vector.tensor_mul(out=vrow[:], in0=mrow[:], in1=mrow[:])
                    nc.vector.tensor_sub(out=vrow[:], in0=erow[:], in1=vrow[:])
                    nc.scalar.activation(out=vrow[:], in_=vrow[:], func=AF.Sqrt,
                                         bias=eps_t[:])
                    nc.vector.reciprocal(out=vrow[:], in_=vrow[:])
                    meanB = stage.tile([128, S], F32, tag="lnB", name=f"meanB{idx}")
                    nc.gpsimd.partition_broadcast(meanB[:], mrow[:])
                    rstdB = stage.tile([128, S], F32, tag="lnB", name=f"rstdB{idx}")
                    nc.gpsimd.partition_broadcast(rstdB[:], vrow[:])
                    for kc in range(KC):
                        dte = stage.tile([128, S], F32, tag="lnd", name=f"lnd{idx}_{kc}")
                        nc.vector.tensor_sub(out=dte[:], in0=resT_t[:, kc, :],
                                             in1=meanB[:])
                        nc.vector.scalar_tensor_tensor(
                            out=dte[:], in0=dte[:], scalar=g_ap[:, kc:kc + 1],
                            in1=rstdB[:], op0=OP.mult, op1=OP.mult)
                        if outT is not None:
                            nc.scalar.activation(out=outT[:, kc, :], in_=dte[:],
                                                 func=AF.Identity,
                                                 bias=b_ap[:, kc:kc + 1])
                        else:
                            ote = stage.tile([128, S], F32, tag="lno",
                                             name=f"lno{idx}_{kc}")
                            nc.scalar.activation(out=ote[:], in_=dte[:],
                                                 func=AF.Identity,
                                                 bias=b_ap[:, kc:kc + 1])
                            nc.sync.dma_start(out=outT_d[:, kc, :], in_=ote[:])

            def emit_ffn():
                """resT_t = relu(o2T @ w1 + b1) @ w2 + b2 + o2T"""
                with tc.tile_pool(name="ffn", bufs=1) as fp, \
                     tc.tile_pool(name="ffn_w", bufs=3) as fw, \
                     tc.tile_pool(name="ffn_w2", bufs=2) as fw2, \
                     tc.tile_pool(name="ffn_ps", bufs=4, space="PSUM") as pf:
                    hT = fp.tile([128, FC, S], F32, tag="hT", name="hT")
                    for mc in range(FC):
                        wt = fw.tile([128, KC, 128], F32, tag="w1t", name=f"w1_{mc}")
                        nc.sync.dma_start(out=wt[:], in_=w1_d[mc, :, :, :])
                        ph = pf.tile([128, S], F32, tag="pf", name=f"ph{mc}")
                        for kc in range(KC):
                            nc.tensor.matmul(ph[:], wt[:, kc, :], o2T_t[:, kc, :],
                                             start=(kc == 0), stop=(kc == KC - 1))
                        nc.scalar.activation(out=hT[:, mc, :], in_=ph[:], func=AF.Relu,
                                             bias=b1_t[:, mc:mc + 1])
                    for mc in range(KC):
                        for half in range(2):
                            wt = fw2.tile([128, FC // 2, 128], F32, tag="w2t",
                                          name=f"w2_{mc}_{half}")
                            nc.sync.dma_start(
                                out=wt[:],
                                in_=w2_d[mc, :, half * (FC // 2):(half + 1) * (FC // 2), :])
                            if half == 0:
                                po = pf.tile([128, S], F32, tag="pf", name=f"po2_{mc}")
                            for k in range(FC // 2):
                                kc = half * (FC // 2) + k
                                nc.tensor.matmul(po[:], wt[:, k, :], hT[:, kc, :],
                                                 start=(kc == 0), stop=(kc == FC - 1))
                        # resT = (po + b2) + o2T
                        nc.vector.scalar_tensor_tensor(
                            out=resT_t[:, mc, :], in0=po[:], scalar=b2_t[:, mc:mc + 1],
                            in1=o2T_t[:, mc, :], op0=OP.add, op1=OP.add)

            # ---------------- main flow ----------------
            with tc.tile_pool(name="maskp", bufs=1) as maskp:
                maskT_t = maskp.tile([128, SC, S], F32, tag="maskT", name="maskT_t")
                nc.sync.dma_start(out=maskT_t[:], in_=maskT_d[:, :, :])

                emit_attention(0, yzT_t, yzT_t, wdec["wq_s"], wdec["wk_s"],
                               wdec["wv_s"], wdec["wo_s"], True, yzT_t)
                emit_ln(0, ln_t["g0"], ln_t["be0"], o1T_t)
                emit_attention(1, o1T_t, xzT_t, wdec["wq_c"], wdec["wk_c"],
                               wdec["wv_c"], wdec["wo_c"], False, o1T_t)
                emit_ln(1, ln_t["g1"], ln_t["be1"], o2T_t)
            emit_ffn()
            emit_ln(2, ln_t["g2"], ln_t["be2"], None)

    nc.compile()
    return nc


def _get_nc():
    if "nc" not in _CACHE:
        _CACHE["nc"] = _build_nc()
    return _CACHE["nc"]


def _stat_blocks(W, mc_n, kc_n):
    """[K, M] weight -> [MC, 128, KC, 128] blocked stationary layout:
    block[mc][p][kc][m] = W[kc*128+p, mc*128+m]."""
    W4 = np.ascontiguousarray(W, dtype=np.float32).reshape(kc_n, 128, mc_n, 128)
    return np.ascontiguousarray(W4.transpose(2, 1, 0, 3))


def _featmaj(x):
    """[T, D] -> [128, D/128, T] feature-major tile layout."""
    xT = np.ascontiguousarray(x, dtype=np.float32).T  # [D, T]
    d, t = xT.shape
    return np.ascontiguousarray(xT.reshape(d // 128, 128, t).transpose(1, 0, 2))


def prepare_inputs(xz, yz, wq_s, wk_s, wv_s, wo_s, wq_c, wk_c, wv_c, wo_c,
                   ffn_w1, ffn_b1, ffn_w2, ffn_b2,
                   ln0_g, ln0_b, ln1_g, ln1_b, ln2_g, ln2_b,
                   look_ahead_mask, pad_mask):
    def headcat(w):  # [H, D, HD] -> [D, H*HD]
        return np.ascontiguousarray(
            np.transpose(np.asarray(w, np.float32), (1, 0, 2)).reshape(D, D))

    shared = {
        "wq_s": _stat_blocks(headcat(wq_s), KC, KC),
        "wk_s": _stat_blocks(headcat(wk_s), KC, KC),
        "wo_s": _stat_blocks(np.asarray(wo_s, np.float32), KC, KC),
        "wq_c": _stat_blocks(headcat(wq_c), KC, KC),
        "wk_c": _stat_blocks(headcat(wk_c), KC, KC),
        "wo_c": _stat_blocks(np.asarray(wo_c, np.float32), KC, KC),
        "wv_s": np.ascontiguousarray(headcat(wv_s).reshape(KC, 128, D)),
        "wv_c": np.ascontiguousarray(headcat(wv_c).reshape(KC, 128, D)),
        "w1": _stat_blocks(np.asarray(ffn_w1, np.float32), FC, KC),
        "w2": _stat_blocks(np.asarray(ffn_w2, np.float32), KC, FC),
        "b1": np.ascontiguousarray(np.asarray(ffn_b1, np.float32).reshape(FC, 128).T),
        "b2": np.ascontiguousarray(np.asarray(ffn_b2, np.float32).reshape(KC, 128).T),
    }
    for nm, g, b in (("0", ln0_g, ln0_b), ("1", ln1_g, ln1_b), ("2", ln2_g, ln2_b)):
        shared["g" + nm] = np.ascontiguousarray(
            np.asarray(g, np.float32).reshape(KC, 128).T)
        shared["be" + nm] = np.ascontiguousarray(
            np.asarray(b, np.float32).reshape(KC, 128).T)
    # additive mask, transposed to [s, t], tiled [128, SC, S]
    m = np.asarray(look_ahead_mask, np.float32)[0, 0]  # [t, s]
    mT = (m.T * NEGBIG).astype(np.float32)             # [s, t]
    shared["maskT"] = np.ascontiguousarray(
        mT.reshape(SC, 128, S).transpose(1, 0, 2))

    in_maps = []
    for c in range(B):
        im = dict(shared)
        im["yzT"] = _featmaj(np.asarray(yz, np.float32)[c])
        im["xzT"] = _featmaj(np.asarray(xz, np.float32)[c])
        pb = (np.asarray(pad_mask, np.float32)[c, 0, 0] * NEGBIG).astype(np.float32)
        im["padb"] = np.ascontiguousarray(pb.reshape(SC, 128).T)
        in_maps.append(im)
    return in_maps


def kernel(**inputs):
    from concourse.bass_utils import run_bass_kernel_spmd

    nc = _get_nc()
    in_maps = prepare_inputs(**inputs)
    res = run_bass_kernel_spmd(nc, in_maps, core_ids=list(range(B)))
    out = np.empty((B, S, D), np.float32)
    for c in range(B):
        oT = res.results[c]["outT"]  # [128, KC, S]
        out[c] = oT.transpose(1, 0, 2).reshape(D, S).T
    return out
